# revision 33
# baseline (speedup 1.0000x reference)
"""Causal self-attention (B=2, T=2048, C=1024, H=16) on 8 trn2 NeuronCores.

Sharding: core c -> (batch b = c // 4, head-group g = c % 4). Each core
computes 4 heads of one batch element end-to-end (qkv slice, causal
attention, its w_proj row-block partial of the output projection).
Host sums the 4 partials per batch and adds b_proj.

Per-core dataflow (all matmul inputs bf16, PSUM accumulation fp32):
  qkT  [n=512, T]  = wqk.T @ x.T         (lhsT = wqk chunks, rhs = xT chunks)
  V    [T, 256+1s] = x @ wv (+ ones col) (lhsT = xT chunks,  rhs = wv)
  per q-half qh, head h, k-tile kj:
    S^T [128, W] = kT_h kj-tile vs qT_h  (K=64 contraction, causal-trimmed)
    P = exp(S^T / 8) -> bf16 sbuf, diagonal block masked by tri mask
    y~aug[65, 512] += Vaug_h[kj].T @ P   (row 64 = softmax denominator)
  y^T = y~ * broadcast(1/denom)          (fp16 selector-matmul broadcast)
  out_partial [T, 1024] = y^T-chunks as lhsT vs wp  (+ host-side bias)

Scheduling: the PE stream is kept dense. y~ matmuls run two k-tiles
behind the S^T matmuls (so the PE never waits on exps); each head-half's
trailing y~ + finalize is carried into the next half; deferred work
(V tiles, remaining qkT tiles, per-q-chunk normalize, projection pieces)
is pumped one item per k-tile into the ACT-bound attention stretches,
enqueued only after its producers are emitted so no engine stream
convoys on a not-yet-ready instruction.
"""

import functools
from collections import deque
from contextlib import ExitStack

import ml_dtypes
import numpy as np

import concourse.bacc as bacc
import concourse.bass as bass
import concourse.mybir as mybir
import concourse.tile as tile
from concourse import bass_utils

BF16 = mybir.dt.bfloat16
F16 = mybir.dt.float16
F32 = mybir.dt.float32
EXP = mybir.ActivationFunctionType.Exp
MULT = mybir.AluOpType.mult

T = 2048
C = 1024
HD = 64
N_CORES = 8
CCHUNK = 8    # contraction chunks of 128 over C
TT = 16       # t-tiles of 128
QC = 4        # q chunks of 512
SCALE = 1.0 / float(np.sqrt(HD))


def build_bass():
    nc = bacc.Bacc("TRN2", target_bir_lowering=False)

    xT_d = nc.dram_tensor("xT", [C, T], BF16, kind="ExternalInput").ap()
    wqk_d = nc.dram_tensor("wqk", [C, 512], BF16, kind="ExternalInput").ap()
    wv_d = nc.dram_tensor("wv", [C, 256], BF16, kind="ExternalInput").ap()
    wp_d = nc.dram_tensor("wp", [256, C], BF16, kind="ExternalInput").ap()
    bqk_d = nc.dram_tensor("bqk", [128, 4], F32, kind="ExternalInput").ap()
    bv_d = nc.dram_tensor("bv", [1, 256], BF16, kind="ExternalInput").ap()
    mask_d = nc.dram_tensor("mask", [128, 128], BF16, kind="ExternalInput").ap()
    ones_d = nc.dram_tensor("ones_bf", [1, 128], BF16, kind="ExternalInput").ap()
    sel_d = nc.dram_tensor("sel", [2, 128], F16, kind="ExternalInput").ap()
    out_d = nc.dram_tensor("out", [T, C], F32, kind="ExternalOutput").ap()

    with tile.TileContext(nc) as tc, ExitStack() as ctx:
        const = ctx.enter_context(tc.tile_pool(name="const", bufs=1))
        xT_sb = const.tile([128, CCHUNK, T], BF16)
        wqk_sb = const.tile([128, CCHUNK, 512], BF16)
        wv_sb = const.tile([128, CCHUNK, 256], BF16)
        wp_sb = const.tile([128, 2, C], BF16)
        bqk_sb = const.tile([128, 4], F32)
        bv_sb = const.tile([1, 256], BF16)
        mask_sb = const.tile([128, 128], BF16)
        ones_sb = const.tile([1, 128], BF16)
        sel_sb = const.tile([2, 128], F16)
        # per-(head-pair, q-chunk) denominator collectors and reciprocals;
        # row within each = h % 2
        colls = {}
        rc32s = {}
        rc16s = {}
        for hp in (0, 1):
            for qc in range(QC):
                colls[(hp, qc)] = const.tile([2, 512], F32, name=f"coll_{hp}_{qc}")
                rc32s[(hp, qc)] = const.tile([2, 512], F32, name=f"rc32_{hp}_{qc}")
                rc16s[(hp, qc)] = const.tile([2, 512], F16, name=f"rc16_{hp}_{qc}")
        qkT_sb = const.tile([128, 4, T], BF16)      # q h01 | q h23 | k h01 | k h23
        vaug_sb = const.tile([128, TT, 4, 65], BF16)
        yT_sb = const.tile([128, 2, T], BF16)

        for cc in range(CCHUNK):
            c0 = 128 * cc
            eng = nc.sync if cc % 2 == 0 else nc.scalar
            eng.dma_start(xT_sb[:, cc, :], xT_d[c0 : c0 + 128, :])
            eng2 = nc.scalar if cc % 2 == 0 else nc.sync
            eng2.dma_start(wqk_sb[:, cc, :], wqk_d[c0 : c0 + 128, :])
        nc.scalar.dma_start(bqk_sb[:, :], bqk_d[:, :])
        nc.scalar.dma_start(mask_sb[:, :], mask_d[:, :])
        for cc in range(CCHUNK):
            eng = nc.sync if cc % 2 == 0 else nc.scalar
            eng.dma_start(wv_sb[:, cc, :], wv_d[128 * cc : 128 * cc + 128, :])
        nc.scalar.dma_start(bv_sb[:, :], bv_d[:, :])
        nc.scalar.dma_start(ones_sb[:, :], ones_d[:, :])
        nc.scalar.dma_start(sel_sb[:, :], sel_d[:, :])
        for dc in range(2):
            nc.sync.dma_start(wp_sb[:, dc, :], wp_d[128 * dc : 128 * dc + 128, :])
        # ones column per (t-tile, head) in the augmented-V layout
        nc.vector.memset(vaug_sb[:, :, :, 64:65], 1.0)

        def qk_ntile(pool, ni, tch):
            """qkT n-tile ni, t-chunk tch: [128, 512] of qkT + bias add."""
            t0 = 512 * tch
            n0 = 128 * ni
            ps = pool.tile([128, 512], F32, tag=pool.name, name=f"psqk_{ni}_{tch}")
            for cc in range(CCHUNK):
                nc.tensor.matmul(
                    ps,
                    lhsT=wqk_sb[:, cc, n0 : n0 + 128],
                    rhs=xT_sb[:, cc, t0 : t0 + 512],
                    start=(cc == 0),
                    stop=(cc == CCHUNK - 1),
                )
            nc.vector.tensor_scalar_add(
                qkT_sb[:, ni, t0 : t0 + 512], ps, bqk_sb[:, ni : ni + 1]
            )

        def v_ttile(pool, tt):
            """V t-tile tt -> vaug columns (with bias via K=1 ones matmul)."""
            ps = pool.tile([128, 256], F32, tag=pool.name, name=f"psv_{tt}")
            for cc in range(CCHUNK):
                nc.tensor.matmul(
                    ps,
                    lhsT=xT_sb[:, cc, 128 * tt : 128 * tt + 128],
                    rhs=wv_sb[:, cc, :],
                    start=(cc == 0),
                    stop=False,
                )
            nc.tensor.matmul(
                ps, lhsT=ones_sb[:, :], rhs=bv_sb[:, :], start=False, stop=True
            )
            nc.vector.tensor_copy(
                vaug_sb[:, tt, :, 0:64], ps.rearrange("p (h e) -> p h e", h=4)
            )

        # ---- phase 1 lead-in: just enough qkT to start h0/h1 attention ----
        with tc.tile_pool(name="pqk", bufs=4, space="PSUM") as pqk:
            for ni, tch in ((2, 0), (0, 0), (0, 1), (2, 1)):
                qk_ntile(pqk, ni, tch)

        # ---- attention (qh-outer), deferred-work queue pumped per k-tile ----
        with tc.tile_pool(name="expp", bufs=6) as epool, \
             tc.tile_pool(name="finp", bufs=10) as fpool, \
             tc.tile_pool(name="outp", bufs=6) as obpool, \
             ExitStack() as psum_ctx:
            spool = psum_ctx.enter_context(
                tc.tile_pool(name="ps_s", bufs=2, space="PSUM"))
            ypool = psum_ctx.enter_context(
                tc.tile_pool(name="ps_y", bufs=2, space="PSUM"))
            paux = psum_ctx.enter_context(
                tc.tile_pool(name="paux", bufs=2, space="PSUM"))

            tasks = deque()
            ysbs = {}

            def pump():
                if tasks:
                    tasks.popleft()()

            def recip_task(hp, qc):
                nc.vector.reciprocal_approx_fast(rc32s[(hp, qc)], colls[(hp, qc)])
                with nc.allow_low_precision(reason="fp16 recip for PE bcast"):
                    nc.vector.tensor_copy(rc16s[(hp, qc)], rc32s[(hp, qc)])

            def bcast_and_mult(h, qc):
                row = h % 2
                pb = 64 * row
                bc = paux.tile([64, 512], F32, tag=paux.name, name=f"bc_{h}_{qc}")
                nc.tensor.matmul(
                    bc,
                    lhsT=sel_sb[:, 64 * row : 64 * row + 64],
                    rhs=rc16s[(h // 2, qc)],
                    start=True,
                    stop=True,
                )
                nc.vector.tensor_tensor(
                    yT_sb[pb : pb + 64, h // 2, 512 * qc : 512 * qc + 512],
                    ysbs[(h, qc)][0:64, :],
                    bc,
                    op=MULT,
                )

            def proj_piece(tt, nch, copy_engine="vector", pool=None):
                pool = pool or paux
                po = pool.tile([128, 512], F32, tag=pool.name, name=f"po_{tt}_{nch}")
                for dc in range(2):
                    nc.tensor.matmul(
                        po,
                        lhsT=yT_sb[:, dc, 128 * tt : 128 * tt + 128],
                        rhs=wp_sb[:, dc, 512 * nch : 512 * nch + 512],
                        start=(dc == 0),
                        stop=(dc == 1),
                    )
                ob = obpool.tile([128, 512], F32, tag="ob", name=f"ob_{tt}_{nch}")
                if copy_engine == "scalar":
                    nc.scalar.copy(ob, po)
                else:
                    nc.vector.tensor_copy(ob, po)
                nc.sync.dma_start(
                    out_d[128 * tt : 128 * tt + 128, 512 * nch : 512 * nch + 512], ob
                )

            def attn_head_half(h, qh, carry=None, on_kj=None):
                """Emit one head's attention over q-half qh. Runs `carry`
                (the previous half's trailing work) after the first k-tile's
                S^T+exp, and returns its own trailing closure."""
                pb = 64 * (h % 2)
                ni_q = h // 2
                ni_k = 2 + h // 2
                qbase = 1024 * qh
                psy = {}
                for qc in (2 * qh, 2 * qh + 1):
                    psy[qc] = ypool.tile([65, 512], F32, tag="y", name=f"psy_{h}_{qc}")

                def emit_y(kj, expS, qlo, qhi, off):
                    for qc in (2 * qh, 2 * qh + 1):
                        lo2 = max(qlo, 512 * qc)
                        hi2 = min(qhi, 512 * qc + 512)
                        if lo2 >= hi2:
                            continue
                        nc.tensor.matmul(
                            psy[qc][:, lo2 - 512 * qc : hi2 - 512 * qc],
                            lhsT=vaug_sb[:, kj, h, 0:65],
                            rhs=expS[:, off + lo2 - qlo : off + hi2 - qlo],
                            start=(kj == 0),
                            stop=(kj == 4 * qc + 3),
                        )

                def finalize_lite(qc):
                    ysb = fpool.tile([65, 512], F32, tag="yf", name=f"yf_{h}_{qc}")
                    nc.vector.tensor_copy(ysb, psy[qc])
                    nc.sync.dma_start(
                        colls[(h // 2, qc)][h % 2 : h % 2 + 1, :], ysb[64:65, :]
                    )
                    ysbs[(h, qc)] = ysb

                def step(kj, expS, qlo, qhi, off):
                    emit_y(kj, expS, qlo, qhi, off)
                    if kj == 4 * (2 * qh) + 3:
                        finalize_lite(2 * qh)
                    elif kj == 4 * (2 * qh + 1) + 3:
                        finalize_lite(2 * qh + 1)

                # greedy-bundle adjacent (small) causal windows into shared
                # PSUM tiles so one ACTIVATE serves several k-tiles
                wins = []
                for kj in range(8 * qh + 8):
                    qlo = max(128 * kj, qbase)
                    wins.append((kj, qlo, qbase + 1024 - qlo))
                bundles = []
                if h == 0 and qh == 0:
                    # split the very first window so the first exp only
                    # needs one q-side qkT lead-in group
                    bundles = [[(0, 0, 512)], [(0, 512, 512)]]
                    wins = wins[1:]
                cur, cap = [], 0
                for w in wins:
                    if cur and cap + w[2] <= 1024:
                        cur.append(w)
                        cap += w[2]
                    else:
                        if cur:
                            bundles.append(cur)
                        cur, cap = [w], w[2]
                bundles.append(cur)

                pend = deque()
                first = True
                for bundle in bundles:
                    total = sum(w for _, _, w in bundle)
                    bkj = bundle[0][0]
                    ps_s = spool.tile(
                        [128, total], F32, tag="s", name=f"pss_{h}_{bkj}_{qh}"
                    )
                    off = 0
                    for kj, qlo, width in bundle:
                        qhi = qlo + width
                        a = qlo
                        while a < qhi:
                            col = off + (a - qlo)
                            stepw = min(qhi - a, 512 - (col % 512))
                            nc.tensor.matmul(
                                ps_s[:, col : col + stepw],
                                lhsT=qkT_sb[pb : pb + 64, ni_k, 128 * kj : 128 * kj + 128],
                                rhs=qkT_sb[pb : pb + 64, ni_q, a : a + stepw],
                                start=True,
                                stop=True,
                            )
                            a += stepw
                        off += width
                    expS = epool.tile(
                        [128, total], BF16, tag="es", name=f"es_{h}_{bkj}_{qh}"
                    )
                    nc.scalar.activation(expS, ps_s, EXP, scale=SCALE)
                    off = 0
                    for kj, qlo, width in bundle:
                        if qlo == 128 * kj:
                            # diagonal block: keep entries with q >= k
                            nc.vector.tensor_tensor(
                                expS[:, off : off + 128],
                                expS[:, off : off + 128],
                                mask_sb,
                                op=MULT,
                            )
                        off += width
                    if first and carry is not None:
                        carry()
                    else:
                        pump()
                        if qh == 0:
                            pump()
                    first = False
                    if on_kj and bkj in on_kj:
                        on_kj[bkj]()
                    off = 0
                    for kj, qlo, width in bundle:
                        pend.append((kj, expS, qlo, qlo + width, off))
                        off += width
                        if len(pend) > 2:
                            step(*pend.popleft())

                def trailing():
                    while pend:
                        step(*pend.popleft())

                return trailing

            # qh0 deferred-work: V tiles and remaining qkT tiles, ordered so
            # each is emitted before its first consumer's head-half.
            for tt in range(0, 8):
                tasks.append(functools.partial(v_ttile, paux, tt))
            for ni, tch in ((3, 0), (3, 1), (1, 0), (1, 1)):
                tasks.append(functools.partial(qk_ntile, paux, ni, tch))
            for tt in range(8, 12):
                tasks.append(functools.partial(v_ttile, paux, tt))
            for ni, tch in ((2, 2), (2, 3), (0, 2), (0, 3)):
                tasks.append(functools.partial(qk_ntile, paux, ni, tch))
            for tt in range(12, 16):
                tasks.append(functools.partial(v_ttile, paux, tt))
            for ni, tch in ((3, 2), (3, 3), (1, 2), (1, 3)):
                tasks.append(functools.partial(qk_ntile, paux, ni, tch))

            def norm_tasks(hp, qc):
                tasks.append(functools.partial(recip_task, hp, qc))
                tasks.append(functools.partial(bcast_and_mult, 2 * hp, qc))
                tasks.append(functools.partial(bcast_and_mult, 2 * hp + 1, qc))

            def carry_plus(prev, fn):
                def f():
                    prev()
                    fn()
                return f

            carry = None
            for h in range(4):
                carry = attn_head_half(h, 0, carry)
                if h == 1:
                    norm_tasks(0, 0)   # h0/h1 qc0 done (finalized at kj4)
                elif h == 2:
                    norm_tasks(0, 1)   # h0/h1 qc1 done (h1 trailing ran in h2)
                elif h == 3:
                    norm_tasks(1, 0)

            # h3's qc1 finalize is inside its trailing; chain the recip after
            carry = carry_plus(carry, functools.partial(recip_task, 1, 1))
            tasks.append(functools.partial(bcast_and_mult, 2, 1))
            tasks.append(functools.partial(bcast_and_mult, 3, 1))
            for tt in range(0, 8):
                for nch in range(2):
                    tasks.append(functools.partial(proj_piece, tt, nch))

            def late_norm12():
                recip_task(1, 2)
                bcast_and_mult(2, 2)
                bcast_and_mult(3, 2)

            for h in range(4):
                hook = {14: late_norm12} if h == 3 else None
                carry = attn_head_half(h, 1, carry, on_kj=hook)
                if h == 1:
                    norm_tasks(0, 2)
                elif h == 2:
                    norm_tasks(0, 3)

            # tail: finish normalize while attention psum pools still open
            carry()            # h3 qh1 trailing (y~ + finalize qc2/qc3)
            while tasks:
                tasks.popleft()()
            recip_task(1, 3)
            bcast_and_mult(2, 3)
            bcast_and_mult(3, 3)
            psum_ctx.close()   # release s/y/aux banks for the projection

            with tc.tile_pool(name="ppo", bufs=6, space="PSUM") as popool:
                k = 0
                for tt in range(8, 16):
                    for nch in range(2):
                        proj_piece(
                            tt, nch,
                            "scalar" if k % 2 == 0 else "vector",
                            pool=popool,
                        )
                        k += 1

    nc.compile()
    return nc


@functools.lru_cache(maxsize=1)
def _bass_cached():
    return build_bass()


def make_in_maps(x, w_attn, b_attn, w_proj):
    bf = ml_dtypes.bfloat16
    mask = np.triu(np.ones((128, 128), np.float32)).astype(bf)
    ones_bf = np.ones((1, 128), bf)
    sel = np.zeros((2, 128), np.float16)
    for i in range(2):
        sel[i, 64 * i : 64 * i + 64] = 1.0
    in_maps = []
    for core in range(N_CORES):
        b, g = core // 4, core % 4
        qs = slice(256 * g, 256 * g + 256)
        ks = slice(1024 + 256 * g, 1024 + 256 * g + 256)
        vs = slice(2048 + 256 * g, 2048 + 256 * g + 256)
        wqk = np.concatenate([w_attn[:, qs], w_attn[:, ks]], axis=1).astype(bf)
        bqk = np.concatenate([b_attn[qs], b_attn[ks]]).astype(np.float32)
        in_maps.append(
            {
                "xT": np.ascontiguousarray(x[b].T).astype(bf),
                "wqk": wqk,
                "wv": np.ascontiguousarray(w_attn[:, vs]).astype(bf),
                "wp": np.ascontiguousarray(
                    w_proj[256 * g : 256 * g + 256, :]
                ).astype(bf),
                "bqk": np.ascontiguousarray(bqk.reshape(4, 128).T),
                "bv": b_attn[vs].reshape(1, 256).astype(bf),
                "mask": mask,
                "ones_bf": ones_bf,
                "sel": sel,
            }
        )
    return in_maps


def run(x, w_attn, b_attn, w_proj, b_proj, trace=False):
    nc = _bass_cached()
    in_maps = make_in_maps(
        np.asarray(x, np.float32),
        np.asarray(w_attn, np.float32),
        np.asarray(b_attn, np.float32),
        np.asarray(w_proj, np.float32),
    )
    res = bass_utils.run_bass_kernel_spmd(
        nc, in_maps, core_ids=list(range(N_CORES)), trace=trace
    )
    out = np.zeros((2, T, C), np.float32)
    for core in range(N_CORES):
        out[core // 4] += res.results[core]["out"]
    out += np.asarray(b_proj, np.float32)[None, None, :]
    return out, res


def kernel(x, w_attn, b_attn, w_proj, b_proj):
    out, _ = run(x, w_attn, b_attn, w_proj, b_proj, trace=False)
    return out
